# revision 67
# baseline (speedup 1.0000x reference)
"""Multi-head attention (B=2, S=2048, D=1024, H=16) on 8 Trainium2 cores.

Sharding: data-parallel over batch (2) x tensor-parallel over head groups (4).
Core c handles batch b = c//4 and heads [g*4, g*4+4) where g = c%4.

v2 dataflow (vs the dh-major baseline; cost model 215us -> 183us, HW
~264us -> ~165us): the attention phase is ACT-bound (exp of 16.8M
scores/core ~= 128us on the activation engine), so the kernel (a) starts
attention as early as possible -- the m0 half of K-proj plus the first
q-chunk's m0 Q-proj stream first and the first exp fires at ~26us; the m1
projections and the whole V projection are woven into the first attention
units' PE slack -- and (b) cuts PE work (176us -> 144us busy) to fit under
the ACT roofline:
  K^T = Wk_g @ x_k^T          (dk on partitions, s free)
  Q^T[:, qc] = Wq_g @ x_q^T[:, qc]
  per (q-chunk 512, head-pair): S^T tiles = K^T_h.T @ Q^T_h, P^T = exp(S^T/8)
  V1 = [V*m | m] per head     (u-outer PSUM groups woven into attention)
  ctx (q-major, all 128 PE columns vs 65 in the baseline): per 128-q subtile
    [ctx | denom][128q, 65] = sum_k P^T[k, q-sub].T @ V1_h[k]
    (the V1 mask column makes column 64 the masked softmax denominator)
  normalize: ctxq = ctx * (1/denom)  (DVE per-partition scalar off PSUM)
  ctx^T via DMA-xbar transpose of [128q x 128(dh pair)] bf16 tiles
  out[qi] = ctx^T.T @ Wo_g^T  (PSUM -> SBUF copy, DMA out)
The last unit is special-cased for the post-exp tail: its ctx chains are
k-rotated to start at the final exp batch, h0/h1 ride the ctx/aux banks,
the transpose runs on the PE (identity matmul), and copies ride the then-
idle ACT engine.

PSUM groups: a bank supports one OPEN accumulation group at a time --
sub-bank groups (ctx subtiles, V pairs, q-proj halves) are emitted
strictly sequentially with skip_group_check=True.

The value bias never reaches the device: sum_k attn*(v+bv) = sum attn*v
+ bv, so the host folds bv @ Wo^T into the output bias.
"""

import numpy as np
import ml_dtypes

import concourse.bass as bass
import concourse.tile as tile
from concourse import bacc, mybir
from concourse.bass_utils import run_bass_kernel_spmd

F32R = mybir.dt.bfloat16
F32 = mybir.dt.float32
EXP = mybir.ActivationFunctionType.Exp

B, S, D = 2, 2048, 1024
HEADS, DK = 16, 64
G = 4                 # head-groups (tensor parallel factor)
HPG = HEADS // G      # 4 heads per group
DH = HPG * DK         # 256 head-dims per group
NCORES = 8
NT = D // 128         # 8 contraction tiles over d_model
NU = S // 128         # 16 s-chunks of 128 (k-position tiles)
NQC = S // 512        # 4 q-chunks of 512

_cached = {}


def _emit(nc, tc, pools, dram, rep):
    (singles, xpool, xqpool, ppool, opool, rpool, big_ps, ctxp, auxp) = pools
    (xkT, xqT, xvT, wqT, wkT, wvT, woT, bq2, bk2, m01, ident, out) = dram

    def resident(name, shape, dt=F32R):
        return singles.tile(shape, dt, tag=name, name=f"{name}_r{rep}")

    wk3 = wkT.rearrange("(t p) d -> p t d", p=128)
    wq3 = wqT.rearrange("(t p) d -> p t d", p=128)
    wv3 = wvT.rearrange("(t p) d -> p t d", p=128)
    wo3 = woT.rearrange("(m p) d -> p m d", p=128)
    xkT3 = xkT.rearrange("(t p) s -> p t s", p=128)
    xqT3 = xqT.rearrange("(t p) s -> p t s", p=128)
    xvT3 = xvT.rearrange("(t p) s -> p t s", p=128)

    # ---- resident tensors ----
    wk_sb = resident("wk_sb", [128, NT, DH])
    wq_sb = resident("wq_sb", [128, NT, DH])
    wv_sb = resident("wv_sb", [128, NT, DH])
    wo_sb = resident("wo_sb", [128, 2, D])
    bq_sb = resident("bq_sb", [128, 2], F32)
    bk_sb = resident("bk_sb", [128, 2], F32)
    m01_sb = resident("m01_sb", [128, NU], F32)
    id_sb = resident("id_sb", [128, 128])
    kT_sb = [resident(f"kT{m}", [128, S]) for m in range(2)]
    qT_sb = [resident(f"qT{m}", [128, S]) for m in range(2)]
    ctxT_sb = [resident(f"ctxT{m}", [128, S]) for m in range(2)]
    v_all = resident("v_all", [128, NU, HPG * 65])
    v4 = v_all.rearrange("p u (h e) -> p u h e", e=65)

    # ---- DMA issue: wk, xk chunks, wq, consts, xq0 first (attention-start
    # critical path); wv/xvp/wo/xq1.. behind them.  Weights+consts ride the
    # scalar (ACT) queue -- all issued before the first exp; x streams ride
    # sync (SP). ----
    xq = {}

    def xq_half_dma(qc, half):
        xt = xqpool.tile([128, NT, 256], F32R, tag="xq",
                         name=f"xq{qc}_{half}_r{rep}")
        nc.sync.dma_start(
            out=xt[:],
            in_=xqT3[:, :, qc * 512 + half * 256:qc * 512 + (half + 1) * 256])
        xq.setdefault(qc, [None, None])[half] = xt

    nc.scalar.dma_start(out=wk_sb[:], in_=wk3)
    xg = []
    for t in range(NT):
        xt = xpool.tile([128, S], F32R, tag="xk", name=f"xk{t}_r{rep}",
                        bufs=8)
        nc.sync.dma_start(out=xt[:], in_=xkT3[:, t, :])
        xg.append(xt)
        if t == 1:
            nc.scalar.dma_start(out=wq_sb[:], in_=wq3)
        elif t == 3:
            nc.scalar.dma_start(out=bq_sb[:], in_=bq2)
            nc.scalar.dma_start(out=bk_sb[:], in_=bk2)
            nc.scalar.dma_start(out=m01_sb[:], in_=m01)
            nc.scalar.dma_start(out=id_sb[:], in_=ident)

    # ---- K projection, m0 half first: the first attention unit (head pair
    # 0) only needs kT/qT[0], so S^T can start before the m1 half exists.
    # xk chunks stay resident for the woven m1 pass. ----
    def kproj_m(m, kbig, ksml):
        for t in range(NT):
            xt = xg[t]
            lhsT = wk_sb[:, t, m * 128:(m + 1) * 128]
            for i in range(3):
                nc.tensor.matmul(
                    kbig[:, i * 512:(i + 1) * 512], lhsT,
                    xt[:, i * 512:(i + 1) * 512],
                    start=(t == 0), stop=(t == NT - 1))
            nc.tensor.matmul(
                ksml[:], lhsT, xt[:, 1536:2048],
                start=(t == 0), stop=(t == NT - 1))
        nc.vector.tensor_scalar_add(
            out=kT_sb[m][:, 0:1536], in0=kbig[:], scalar1=bk_sb[:, m:m + 1])
        nc.vector.tensor_scalar_add(
            out=kT_sb[m][:, 1536:2048], in0=ksml[:],
            scalar1=bk_sb[:, m:m + 1])

    def qproj_m(qc, m):
        qp = auxp.tile([128, 512], F32, tag="aux", name=f"qp{qc}_{m}_r{rep}")
        for half in range(2):       # one PSUM group per half, sequential
            for t in range(NT):
                nc.tensor.matmul(
                    qp[:, half * 256:(half + 1) * 256],
                    wq_sb[:, t, m * 128:(m + 1) * 128],
                    xq[qc][half][:, t, :],
                    start=(t == 0), stop=(t == NT - 1),
                    skip_group_check=True)
        nc.vector.tensor_scalar_add(
            out=qT_sb[m][:, qc * 512:(qc + 1) * 512], in0=qp[:],
            scalar1=bq_sb[:, m:m + 1])

    def qproj(qc):
        for m in range(2):
            qproj_m(qc, m)

    def xq_dma(qc):
        xq_half_dma(qc, 0)
        xq_half_dma(qc, 1)

    # ---- head: K-proj m0 chunks 0-6, then the m0 Q-proj halves woven
    # around chunk 7 (their aux PSUM group lives in a different bank, so
    # the PE stays fed while the last xk chunk is still in flight) ----
    xq_dma(0)
    kb0 = big_ps.tile([128, 1536], F32, tag="big", name=f"kb0_r{rep}")
    ks0 = ctxp.tile([128, 512], F32, tag="ctx", name=f"ks0_r{rep}")

    def km0_chunk(t):
        lhsT = wk_sb[:, t, 0:128]
        for i in range(3):
            nc.tensor.matmul(
                kb0[:, i * 512:(i + 1) * 512], lhsT,
                xg[t][:, i * 512:(i + 1) * 512],
                start=(t == 0), stop=(t == NT - 1))
        nc.tensor.matmul(
            ks0[:], lhsT, xg[t][:, 1536:2048],
            start=(t == 0), stop=(t == NT - 1))

    qp00 = auxp.tile([128, 512], F32, tag="aux", name=f"qp0_0_r{rep}")

    def qp00_half(half):
        for t in range(NT):
            nc.tensor.matmul(
                qp00[:, half * 256:(half + 1) * 256],
                wq_sb[:, t, 0:128], xq[0][half][:, t, :],
                start=(t == 0), stop=(t == NT - 1), skip_group_check=True)

    for t in range(NT - 1):
        km0_chunk(t)
    qp00_half(0)
    km0_chunk(NT - 1)
    qp00_half(1)
    nc.vector.tensor_scalar_add(
        out=kT_sb[0][:, 0:1536], in0=kb0[:], scalar1=bk_sb[:, 0:1])
    nc.vector.tensor_scalar_add(
        out=kT_sb[0][:, 1536:2048], in0=ks0[:], scalar1=bk_sb[:, 0:1])
    nc.vector.tensor_scalar_add(
        out=qT_sb[0][:, 0:512], in0=qp00[:], scalar1=bq_sb[:, 0:1])

    _km1 = {}

    def km1_half(j, half):
        # m1 K-projection in 256-wide groups through the aux bank (the big
        # slots belong to the S^T pipeline by now); half-size work items so
        # the weave never delays an S^T batch by more than ~1us
        if half == 0:
            _km1[j] = auxp.tile([128, 512], F32, tag="aux",
                                name=f"km1g{j}_r{rep}")
        kp = _km1[j]
        for t in range(NT):
            nc.tensor.matmul(
                kp[:, half * 256:(half + 1) * 256],
                wk_sb[:, t, 128:256],
                xg[t][:, j * 512 + half * 256:j * 512 + (half + 1) * 256],
                start=(t == 0), stop=(t == NT - 1),
                skip_group_check=True)
        if half == 1:
            nc.vector.tensor_scalar_add(
                out=kT_sb[1][:, j * 512:(j + 1) * 512], in0=kp[:],
                scalar1=bk_sb[:, 1:2])

    # V inputs: per-pair column DMAs (land during the first attention units)
    nc.scalar.dma_start(out=wv_sb[:], in_=wv3)
    nc.scalar.dma_start(out=wo_sb[:], in_=wo3)
    xvp = []
    for p in range(8):
        xt = xpool.tile([128, NT, 256], F32R, tag="xv", name=f"xv{p}_r{rep}")
        nc.sync.dma_start(out=xt[:], in_=xvT3[:, :, p * 256:(p + 1) * 256])
        xvp.append(xt)
    # mask columns of V1 (builds softmax denominators in the ctx matmuls)
    for h in range(HPG):
        nc.vector.tensor_copy(
            out=v4[:, :, h, 64:65],
            in_=m01_sb[:].rearrange("p (u o) -> p u o", o=1))

    def vpair(p):
        # V1[:, 2p:2p+2] = [V*m | m]: u-outer projection pair; even pairs use
        # the ctx PSUM slot, odd pairs the aux slot (parallel pipelines).
        pool, tag = (ctxp, "ctx") if p % 2 == 0 else (auxp, "aux")
        cv = pool.tile([128, 512], F32, tag=tag, name=f"vp{p}_r{rep}")
        # groups must be sequential: a PSUM bank supports one OPEN
        # accumulation group at a time
        for j in range(2):
            for t in range(NT):
                nc.tensor.matmul(
                    cv[:, j * 256:(j + 1) * 256],
                    xvp[p][:, t, j * 128:(j + 1) * 128], wv_sb[:, t, :],
                    start=(t == 0), stop=(t == NT - 1),
                    skip_group_check=True)
        for j in range(2):
            u = p * 2 + j
            nc.vector.tensor_scalar_mul(
                out=v4[:, u, :, 0:64],
                in0=cv[:, j * 256:(j + 1) * 256].rearrange(
                    "p (h e) -> p h e", e=64),
                scalar1=m01_sb[:, u:u + 1])

    # ---- attention producer: software-pipelined over (qc, head-PAIR) ----
    state = {}

    def attn_produce(qc, pr):
        m = pr
        qsl = slice(qc * 512, (qc + 1) * 512)
        pt = {}
        for hh in range(2):
            h = pr * 2 + hh
            pt[h] = [ppool.tile([128, 4096], F32R, tag="pt",
                                name=f"pt{qc}_{h}_{half}_r{rep}")
                     for half in range(2)]
        state[(qc, pr)] = pt
        batches = [(half, b0, bsz) for half in range(2)
                   for (b0, bsz) in ((0, 3), (3, 3), (6, 2))]

        def emit_batch(i):
            half, b0, bsz = batches[i]
            sts = [big_ps.tile([128, bsz * 512], F32, tag="big",
                               name=f"st{qc}_{pr}_{half}_{b0}_{hh}_r{rep}")
                   for hh in range(2)]
            for j in range(bsz):
                k = half * 8 + b0 + j
                for hh in range(2):
                    roff = hh * 64
                    nc.tensor.matmul(
                        sts[hh][:, j * 512:(j + 1) * 512],
                        kT_sb[m][roff:roff + 64, k * 128:(k + 1) * 128],
                        qT_sb[m][roff:roff + 64, qsl],
                        start=True, stop=True)
            for hh in range(2):
                h = pr * 2 + hh
                nc.scalar.activation(
                    out=pt[h][half][:, b0 * 512:(b0 + bsz) * 512],
                    in_=sts[hh][:, 0:bsz * 512], func=EXP, scale=0.125)
        return emit_batch

    # ---- consumer: q-major ctx + denom, DVE normalize, DMA-xbar ctx^T ----
    def cons_tasks(qc, pr):
        pt = state[(qc, pr)]
        qtiles = [rpool.tile([128, 128], F32R, tag="cq",
                             name=f"cq{qc}_{pr}_{q}_r{rep}", bufs=4)
                  for q in range(4)]
        tasks = []

        def do_qsub(hh, qsub, ctile):
            h = pr * 2 + hh
            for k in range(NU):
                nc.tensor.matmul(
                    ctile[:, qsub * 128:qsub * 128 + 65],
                    pt[h][k // 8][:, (k % 8) * 512 + qsub * 128:
                                  (k % 8) * 512 + qsub * 128 + 128],
                    v_all[:, k, h * 65:(h + 1) * 65],
                    start=(k == 0), stop=(k == NU - 1),
                    skip_group_check=True)
            rec = rpool.tile([128, 1], F32, tag="rec",
                             name=f"rc{qc}_{h}_{qsub}_r{rep}", bufs=4)
            nc.vector.reciprocal(
                out=rec[:], in_=ctile[:, qsub * 128 + 64:qsub * 128 + 65])
            nc.vector.tensor_scalar_mul(
                out=qtiles[qsub][:, hh * 64:(hh + 1) * 64],
                in0=ctile[:, qsub * 128:qsub * 128 + 64], scalar1=rec[:])

        def do_head(hh):
            ctile = ctxp.tile([128, 512], F32, tag="ctx",
                              name=f"ct{qc}_{pr}_{hh}_r{rep}")
            out_tasks = []
            for qsub in range(4):
                out_tasks.append(lambda h_=hh, q_=qsub, c_=ctile:
                                 do_qsub(h_, q_, c_))
            return out_tasks

        for hh in range(2):
            tasks.extend(do_head(hh))

        def do_transposes():
            for qsub in range(4):
                qi = qc * 4 + qsub
                nc.sync.dma_start_transpose(
                    out=ctxT_sb[pr][:, qi * 128:(qi + 1) * 128],
                    in_=qtiles[qsub][:])
        tasks.append(do_transposes)
        return tasks

    def oproj_emit(qc, sc, nj, o_sb):
        qi = qc * 4 + sc
        ops = auxp.tile([128, 512], F32, tag="aux", name=f"op{qi}_{nj}_r{rep}")
        for m_ in range(2):
            nc.tensor.matmul(
                ops[:], ctxT_sb[m_][:, qi * 128:(qi + 1) * 128],
                wo_sb[:, m_, nj * 512:(nj + 1) * 512],
                start=(m_ == 0), stop=(m_ == 1))
        nc.vector.tensor_copy(
            out=o_sb[:, nj * 512:(nj + 1) * 512], in_=ops[:])
        if nj == 1:
            nc.sync.dma_start(out=out[qi * 128:(qi + 1) * 128, :], in_=o_sb[:])

    def oproj_tasks(qc):
        tasks = []
        for sc in range(4):
            box = {}

            def nj0(q_=qc, s=sc, b=box):
                b["o"] = opool.tile([128, D], F32, tag="out",
                                    name=f"o{q_}_{s}_r{rep}")
                oproj_emit(q_, s, 0, b["o"])

            def nj1(q_=qc, s=sc, b=box):
                oproj_emit(q_, s, 1, b["o"])
            tasks += [nj0, nj1]
        return tasks

    # ---- main pipeline: window ui produces unit ui, weaving V-pairs
    # (ui 0-2), consumption of unit ui-2, and dripped o-proj work into the
    # producer's batch gaps.  All V-pairs are emitted before the first cons
    # chunk (its k-loop reads every V column). ----
    units = [(qc, pr) for qc in range(NQC) for pr in range(2)]
    vq = [lambda p_=p: vpair(p_) for p in range(8)]
    odue = []
    for ui, (qc, pr) in enumerate(units):
        emit_batch = attn_produce(qc, pr)
        work = []
        if ui == 0:
            work += [lambda j_=j, h_=h: km1_half(j_, h_)
                     for j in range(4) for h in range(2)]
            # xq DMAs reuse slots: emit each only after the previous qc's
            # q-proj reads are on the books
            work += [lambda: qproj_m(0, 1), lambda: xq_dma(1)]
            work += vq[0:2]
        elif ui == 1:
            work += vq[2:7]
            work += [lambda: qproj(1)]
        elif ui == 2:
            work += [lambda: xq_dma(2)] + vq[7:8]
        elif ui in (3, 5):
            nqc = (ui + 1) // 2
            work += [lambda m_=m, q_=nqc: qproj_m(q_, m_)
                     for m in range(2)]
            if nqc == 2:
                work += [lambda: xq_dma(3)]
        if ui >= 2:
            work += cons_tasks(*units[ui - 2])
        per = (len(work) + 5) // 6 if work else 0
        for i in range(6):
            emit_batch(i)
            if per:
                for fn in work[i * per:(i + 1) * per]:
                    fn()
            if odue:
                odue.pop(0)()
        if ui >= 2 and units[ui - 2][1] == 1:
            odue += oproj_tasks(units[ui - 2][0])

    # ---- drain: consume unit 6 against the remaining o-proj backlog ----
    for ti, fn in enumerate(cons_tasks(*units[6])):
        fn()
        if odue:
            odue.pop(0)()
    for fn in odue:
        fn()

    # ---- final unit: per-q-subtile pipeline so the post-exp tail is one
    # subtile deep, not a whole unit.  h1's ctx PSUM rides a big slot (the
    # S^T pipeline is done with it). ----
    qc, pr = units[7]
    pt = state[(qc, pr)]
    # h0 ctx in the ctx bank, h1 in aux: both banks free well before the
    # last exp, so all k<14 accumulation happens during exp(u7).  The big
    # slots only free at the very end -- they host the (post-exp) transpose
    # and o-proj tiles instead, alternating so copies overlap.
    ct = [ctxp.tile([128, 512], F32, tag="ctx", name=f"ctf0_r{rep}"),
          auxp.tile([128, 512], F32, tag="aux", name=f"ctf1_r{rep}")]
    korder = [14, 15] + list(range(14))   # last-exp'd chunks first: each
    for qsub in range(4):                 # chain starts right at the gate
        qi = qc * 4 + qsub
        qtile = rpool.tile([128, 128], F32R, tag="cq",
                           name=f"cqf{qsub}_r{rep}", bufs=4)
        for hh in range(2):
            h = pr * 2 + hh
            for ki, k in enumerate(korder):
                nc.tensor.matmul(
                    ct[hh][:, qsub * 128:qsub * 128 + 65],
                    pt[h][k // 8][:, (k % 8) * 512 + qsub * 128:
                                  (k % 8) * 512 + qsub * 128 + 128],
                    v_all[:, k, h * 65:(h + 1) * 65],
                    start=(ki == 0), stop=(ki == NU - 1),
                    skip_group_check=True)
        for hh in range(2):
            h = pr * 2 + hh
            rec = rpool.tile([128, 1], F32, tag="rec",
                             name=f"rcf{h}_{qsub}_r{rep}", bufs=4)
            nc.vector.reciprocal(
                out=rec[:], in_=ct[hh][:, qsub * 128 + 64:qsub * 128 + 65])
            nc.vector.tensor_scalar_mul(
                out=qtile[:, hh * 64:(hh + 1) * 64],
                in0=ct[hh][:, qsub * 128:qsub * 128 + 64], scalar1=rec[:])
        # PE transpose (latency ~0.7us vs ~2.5us DMA-xbar roundtrip): the
        # S^T pipeline is done, so its PSUM slots are free for the dest
        tp = big_ps.tile([128, 128], F32R, tag="big", name=f"tp{qsub}_r{rep}")
        nc.tensor.matmul(tp[:], qtile[:], id_sb[:], is_transpose=True)
        # tail PSUM->SBUF copies ride the ACT engine (idle after the last
        # exp) so the DVE only carries the normalizations here
        nc.scalar.activation(
            out=ctxT_sb[pr][:, qi * 128:(qi + 1) * 128], in_=tp[:],
            func=mybir.ActivationFunctionType.Copy)
        o_sb = opool.tile([128, D], F32, tag="out", name=f"o3f_{qsub}_r{rep}")
        for nj in range(2):
            ops = big_ps.tile([128, 512], F32, tag="big",
                              name=f"opf{qi}_{nj}_r{rep}")
            for m_ in range(2):
                nc.tensor.matmul(
                    ops[:], ctxT_sb[m_][:, qi * 128:(qi + 1) * 128],
                    wo_sb[:, m_, nj * 512:(nj + 1) * 512],
                    start=(m_ == 0), stop=(m_ == 1))
            nc.scalar.activation(
                out=o_sb[:, nj * 512:(nj + 1) * 512], in_=ops[:],
                func=mybir.ActivationFunctionType.Copy)
            nc.sync.dma_start(
                out=out[qi * 128:(qi + 1) * 128, nj * 512:(nj + 1) * 512],
                in_=o_sb[:, nj * 512:(nj + 1) * 512])


def _build_program(reps=1):
    nc = bacc.Bacc("TRN2", target_bir_lowering=False, debug=False,
                   num_devices=NCORES)

    xkT = nc.dram_tensor("xkT", [D, S], F32R, kind="ExternalInput").ap()
    xqT = nc.dram_tensor("xqT", [D, S], F32R, kind="ExternalInput").ap()
    xvT = nc.dram_tensor("xvT", [D, S], F32R, kind="ExternalInput").ap()
    wqT = nc.dram_tensor("wqT", [D, DH], F32R, kind="ExternalInput").ap()
    wkT = nc.dram_tensor("wkT", [D, DH], F32R, kind="ExternalInput").ap()
    wvT = nc.dram_tensor("wvT", [D, DH], F32R, kind="ExternalInput").ap()
    woT = nc.dram_tensor("woT", [DH, D], F32R, kind="ExternalInput").ap()
    bq2 = nc.dram_tensor("bq2", [128, 2], F32, kind="ExternalInput").ap()
    bk2 = nc.dram_tensor("bk2", [128, 2], F32, kind="ExternalInput").ap()
    m01 = nc.dram_tensor("m01", [128, NU], F32, kind="ExternalInput").ap()
    ident = nc.dram_tensor("ident", [128, 128], F32R,
                           kind="ExternalInput").ap()
    out = nc.dram_tensor("out", [S, D], F32, kind="ExternalOutput").ap()
    dram = (xkT, xqT, xvT, wqT, wkT, wvT, woT, bq2, bk2, m01, ident, out)

    with tile.TileContext(nc) as tc:
        with (
            nc.allow_low_precision(
                reason="bf16 SBUF tiles; the PE truncates to fp22 at "
                       "multiply regardless"),
            tc.tile_pool(name="singles", bufs=1) as singles,
            tc.tile_pool(name="xpool", bufs=2) as xpool,
            tc.tile_pool(name="xqpool", bufs=2) as xqpool,
            tc.tile_pool(name="ppool", bufs=12) as ppool,
            tc.tile_pool(name="opool", bufs=2) as opool,
            tc.tile_pool(name="rpool", bufs=4) as rpool,
            tc.tile_pool(name="big_ps", bufs=2, space="PSUM") as big_ps,
            tc.tile_pool(name="ctx_ps", bufs=1, space="PSUM") as ctxp,
            tc.tile_pool(name="aux_ps", bufs=1, space="PSUM") as auxp,
        ):
            pools = (singles, xpool, xqpool, ppool, opool, rpool, big_ps,
                     ctxp, auxp)
            for rep in range(reps):
                _emit(nc, tc, pools, dram, rep)

    nc.compile()
    return nc


def _get_program():
    if "nc" not in _cached:
        _cached["nc"] = _build_program()
    return _cached["nc"]


def kernel(query, key, value, mask, Wq, bq, Wk, bk, Wv, bv, Wo, bo):
    query = np.asarray(query, dtype=np.float32)
    key = np.asarray(key, dtype=np.float32)
    value = np.asarray(value, dtype=np.float32)
    mask = np.asarray(mask)
    Wq, bq = np.asarray(Wq, dtype=np.float32), np.asarray(bq, dtype=np.float32)
    Wk, bk = np.asarray(Wk, dtype=np.float32), np.asarray(bk, dtype=np.float32)
    Wv, bv = np.asarray(Wv, dtype=np.float32), np.asarray(bv, dtype=np.float32)
    Wo, bo = np.asarray(Wo, dtype=np.float32), np.asarray(bo, dtype=np.float32)

    nc = _get_program()

    c = np.ascontiguousarray
    in_maps = []
    for core in range(NCORES):
        b, g = core // G, core % G
        sl = slice(g * DH, (g + 1) * DH)
        mk = (mask[b, 0, 0, :] != 0).astype(np.float32)
        bf = ml_dtypes.bfloat16
        in_maps.append({
            "xqT": c(query[b].T).astype(bf), "xkT": c(key[b].T).astype(bf),
            "xvT": c(value[b].T).astype(bf),
            "wqT": c(Wq[sl, :].T).astype(bf), "wkT": c(Wk[sl, :].T).astype(bf),
            "wvT": c(Wv[sl, :].T).astype(bf),
            "woT": c(Wo[:, sl].T).astype(bf),
            "bq2": c(bq[sl].reshape(2, 128).T), "bk2": c(bk[sl].reshape(2, 128).T),
            "m01": c(mk.reshape(NU, 128).T),
            "ident": np.eye(128, dtype=bf),
        })

    res = run_bass_kernel_spmd(nc, in_maps, core_ids=list(range(NCORES)))
    _cached["last_results"] = res

    # value-bias folds into the output bias: sum_k attn*(v+bv) = ctx + bv
    bo_eff = bo + bv @ Wo.T
    result = np.empty((B, S, D), dtype=np.float32)
    for b in range(B):
        acc = res.results[b * G + 0]["out"].copy()
        for g in range(1, G):
            acc += res.results[b * G + g]["out"]
        result[b] = acc + bo_eff
    return result


# revision 71
# speedup vs baseline: 1.2301x; 1.2301x over previous
"""Multi-head attention (B=2, S=2048, D=1024, H=16) on 8 Trainium2 cores.

Sharding: data-parallel over batch (2) x tensor-parallel over head groups (4).
Core c handles batch b = c//4 and heads [g*4, g*4+4) where g = c%4.

v2 dataflow (vs the dh-major baseline; cost model 215us -> 183us, HW
~264us -> ~165us): the attention phase is ACT-bound (exp of 16.8M
scores/core ~= 128us on the activation engine), so the kernel (a) starts
attention as early as possible -- the m0 half of K-proj plus the first
q-chunk's m0 Q-proj stream first and the first exp fires at ~26us; the m1
projections and the whole V projection are woven into the first attention
units' PE slack -- and (b) cuts PE work (176us -> 144us busy) to fit under
the ACT roofline:
  K^T = Wk_g @ x_k^T          (dk on partitions, s free)
  Q^T[:, qc] = Wq_g @ x_q^T[:, qc]
  per (q-chunk 512, head-pair): S^T tiles = K^T_h.T @ Q^T_h, P^T = exp(S^T/8)
  V1 = [V*m | m] per head     (u-outer PSUM groups woven into attention)
  ctx (q-major, all 128 PE columns vs 65 in the baseline): per 128-q subtile
    [ctx | denom][128q, 65] = sum_k P^T[k, q-sub].T @ V1_h[k]
    (the V1 mask column makes column 64 the masked softmax denominator)
  normalize: ctxq = ctx * (1/denom)  (DVE per-partition scalar off PSUM)
  ctx^T via DMA-xbar transpose of [128q x 128(dh pair)] bf16 tiles
  out[qi] = ctx^T.T @ Wo_g^T  (PSUM -> SBUF copy, DMA out)
The last unit is special-cased for the post-exp tail: its ctx chains are
k-rotated to start at the final exp batch, h0/h1 ride the ctx/aux banks,
the transpose runs on the PE (identity matmul), and copies ride the then-
idle ACT engine.

PSUM groups: a bank supports one OPEN accumulation group at a time --
sub-bank groups (ctx subtiles, V pairs, q-proj halves) are emitted
strictly sequentially with skip_group_check=True.

The value bias never reaches the device: sum_k attn*(v+bv) = sum attn*v
+ bv, so the host folds bv @ Wo^T into the output bias.
"""

import numpy as np
import ml_dtypes

import concourse.bass as bass
import concourse.tile as tile
from concourse import bacc, mybir
from concourse.bass_utils import run_bass_kernel_spmd

F32R = mybir.dt.bfloat16
F32 = mybir.dt.float32
EXP = mybir.ActivationFunctionType.Exp

B, S, D = 2, 2048, 1024
HEADS, DK = 16, 64
G = 4                 # head-groups (tensor parallel factor)
HPG = HEADS // G      # 4 heads per group
DH = HPG * DK         # 256 head-dims per group
NCORES = 8
NT = D // 128         # 8 contraction tiles over d_model
NU = S // 128         # 16 s-chunks of 128 (k-position tiles)
NQC = S // 512        # 4 q-chunks of 512

_cached = {}


def _emit(nc, tc, pools, dram, rep):
    (singles, xpool, xqpool, ppool, opool, rpool, big_ps, ctxp, auxp) = pools
    (xkT, xqT, xvT, wqT, wkT, wvT, woT, bq2, bk2, m01, ident, out) = dram

    def resident(name, shape, dt=F32R):
        return singles.tile(shape, dt, tag=name, name=f"{name}_r{rep}")

    wk3 = wkT.rearrange("(t p) d -> p t d", p=128)
    wq3 = wqT.rearrange("(t p) d -> p t d", p=128)
    wv3 = wvT.rearrange("(t p) d -> p t d", p=128)
    wo3 = woT.rearrange("(m p) d -> p m d", p=128)
    xkT3 = xkT.rearrange("(t p) s -> p t s", p=128)
    xqT3 = xqT.rearrange("(t p) s -> p t s", p=128)
    xvT3 = xvT.rearrange("(t p) s -> p t s", p=128)

    # ---- resident tensors ----
    wk_sb = resident("wk_sb", [128, NT, DH])
    wq_sb = resident("wq_sb", [128, NT, DH])
    wv_sb = resident("wv_sb", [128, NT, DH])
    wo_sb = resident("wo_sb", [128, 2, D])
    bq_sb = resident("bq_sb", [128, 2], F32)
    bk_sb = resident("bk_sb", [128, 2], F32)
    m01_sb = resident("m01_sb", [128, NU], F32)
    id_sb = resident("id_sb", [128, 128])
    kT_sb = [resident(f"kT{m}", [128, S]) for m in range(2)]
    qT_sb = [resident(f"qT{m}", [128, S]) for m in range(2)]
    ctxT_sb = [resident(f"ctxT{m}", [128, S]) for m in range(2)]
    v_all = resident("v_all", [128, NU, HPG * 65])
    v4 = v_all.rearrange("p u (h e) -> p u h e", e=65)

    # ---- DMA issue: wk, xk chunks, wq, consts, xq0 first (attention-start
    # critical path); wv/xvp/wo/xq1.. behind them.  Weights+consts ride the
    # scalar (ACT) queue -- all issued before the first exp; x streams ride
    # sync (SP). ----
    xq = {}

    def xq_half_dma(qc, half):
        xt = xqpool.tile([128, NT, 256], F32R, tag="xq",
                         name=f"xq{qc}_{half}_r{rep}")
        nc.sync.dma_start(
            out=xt[:],
            in_=xqT3[:, :, qc * 512 + half * 256:qc * 512 + (half + 1) * 256])
        xq.setdefault(qc, [None, None])[half] = xt

    nc.scalar.dma_start(out=wk_sb[:], in_=wk3)
    xg = []
    for t in range(NT):
        xt = xpool.tile([128, S], F32R, tag="xk", name=f"xk{t}_r{rep}",
                        bufs=8)
        nc.sync.dma_start(out=xt[:], in_=xkT3[:, t, :])
        xg.append(xt)
        if t == 1:
            nc.scalar.dma_start(out=wq_sb[:], in_=wq3)
        elif t == 3:
            nc.scalar.dma_start(out=bq_sb[:], in_=bq2)
            nc.scalar.dma_start(out=bk_sb[:], in_=bk2)
            nc.scalar.dma_start(out=m01_sb[:], in_=m01)
            nc.scalar.dma_start(out=id_sb[:], in_=ident)

    # ---- K projection, m0 half first: the first attention unit (head pair
    # 0) only needs kT/qT[0], so S^T can start before the m1 half exists.
    # xk chunks stay resident for the woven m1 pass. ----
    def kproj_m(m, kbig, ksml):
        for t in range(NT):
            xt = xg[t]
            lhsT = wk_sb[:, t, m * 128:(m + 1) * 128]
            for i in range(3):
                nc.tensor.matmul(
                    kbig[:, i * 512:(i + 1) * 512], lhsT,
                    xt[:, i * 512:(i + 1) * 512],
                    start=(t == 0), stop=(t == NT - 1))
            nc.tensor.matmul(
                ksml[:], lhsT, xt[:, 1536:2048],
                start=(t == 0), stop=(t == NT - 1))
        nc.vector.tensor_scalar_add(
            out=kT_sb[m][:, 0:1536], in0=kbig[:], scalar1=bk_sb[:, m:m + 1])
        nc.vector.tensor_scalar_add(
            out=kT_sb[m][:, 1536:2048], in0=ksml[:],
            scalar1=bk_sb[:, m:m + 1])

    def qproj_m(qc, m):
        qp = auxp.tile([128, 512], F32, tag="aux", name=f"qp{qc}_{m}_r{rep}")
        for half in range(2):       # one PSUM group per half, sequential
            for t in range(NT):
                nc.tensor.matmul(
                    qp[:, half * 256:(half + 1) * 256],
                    wq_sb[:, t, m * 128:(m + 1) * 128],
                    xq[qc][half][:, t, :],
                    start=(t == 0), stop=(t == NT - 1),
                    skip_group_check=True)
        nc.vector.tensor_scalar_add(
            out=qT_sb[m][:, qc * 512:(qc + 1) * 512], in0=qp[:],
            scalar1=bq_sb[:, m:m + 1])

    def qproj(qc):
        for m in range(2):
            qproj_m(qc, m)

    def xq_dma(qc):
        xq_half_dma(qc, 0)
        xq_half_dma(qc, 1)

    # ---- head: K-proj m0 chunks 0-6, then the m0 Q-proj halves woven
    # around chunk 7 (their aux PSUM group lives in a different bank, so
    # the PE stays fed while the last xk chunk is still in flight) ----
    xq_dma(0)
    kb0 = big_ps.tile([128, 1536], F32, tag="big", name=f"kb0_r{rep}")
    ks0 = ctxp.tile([128, 512], F32, tag="ctx", name=f"ks0_r{rep}")

    def km0_chunk(t):
        lhsT = wk_sb[:, t, 0:128]
        for i in range(3):
            nc.tensor.matmul(
                kb0[:, i * 512:(i + 1) * 512], lhsT,
                xg[t][:, i * 512:(i + 1) * 512],
                start=(t == 0), stop=(t == NT - 1))
        nc.tensor.matmul(
            ks0[:], lhsT, xg[t][:, 1536:2048],
            start=(t == 0), stop=(t == NT - 1))

    qp00 = auxp.tile([128, 512], F32, tag="aux", name=f"qp0_0_r{rep}")

    def qp00_half(half):
        for t in range(NT):
            nc.tensor.matmul(
                qp00[:, half * 256:(half + 1) * 256],
                wq_sb[:, t, 0:128], xq[0][half][:, t, :],
                start=(t == 0), stop=(t == NT - 1), skip_group_check=True)

    for t in range(NT - 1):
        km0_chunk(t)
    qp00_half(0)
    km0_chunk(NT - 1)
    qp00_half(1)
    nc.vector.tensor_scalar_add(
        out=kT_sb[0][:, 0:1536], in0=kb0[:], scalar1=bk_sb[:, 0:1])
    nc.vector.tensor_scalar_add(
        out=kT_sb[0][:, 1536:2048], in0=ks0[:], scalar1=bk_sb[:, 0:1])
    nc.vector.tensor_scalar_add(
        out=qT_sb[0][:, 0:512], in0=qp00[:], scalar1=bq_sb[:, 0:1])

    _km1 = {}

    def km1_half(j, half):
        # m1 K-projection in 256-wide groups through the aux bank (the big
        # slots belong to the S^T pipeline by now); half-size work items so
        # the weave never delays an S^T batch by more than ~1us
        if half == 0:
            _km1[j] = auxp.tile([128, 512], F32, tag="aux",
                                name=f"km1g{j}_r{rep}")
        kp = _km1[j]
        for t in range(NT):
            nc.tensor.matmul(
                kp[:, half * 256:(half + 1) * 256],
                wk_sb[:, t, 128:256],
                xg[t][:, j * 512 + half * 256:j * 512 + (half + 1) * 256],
                start=(t == 0), stop=(t == NT - 1),
                skip_group_check=True)
        if half == 1:
            nc.vector.tensor_scalar_add(
                out=kT_sb[1][:, j * 512:(j + 1) * 512], in0=kp[:],
                scalar1=bk_sb[:, 1:2])

    # V inputs: per-pair column DMAs (land during the first attention units)
    nc.scalar.dma_start(out=wv_sb[:], in_=wv3)
    nc.scalar.dma_start(out=wo_sb[:], in_=wo3)
    xvp = []
    for p in range(8):
        xt = xpool.tile([128, NT, 256], F32R, tag="xv", name=f"xv{p}_r{rep}")
        nc.sync.dma_start(out=xt[:], in_=xvT3[:, :, p * 256:(p + 1) * 256])
        xvp.append(xt)
    # mask columns of V1 (builds softmax denominators in the ctx matmuls)
    for h in range(HPG):
        nc.vector.tensor_copy(
            out=v4[:, :, h, 64:65],
            in_=m01_sb[:].rearrange("p (u o) -> p u o", o=1))

    def vpair(p):
        # V1[:, 2p:2p+2] = [V*m | m]: u-outer projection pair; even pairs use
        # the ctx PSUM slot, odd pairs the aux slot (parallel pipelines).
        pool, tag = (ctxp, "ctx") if p % 2 == 0 else (auxp, "aux")
        cv = pool.tile([128, 512], F32, tag=tag, name=f"vp{p}_r{rep}")
        # groups must be sequential: a PSUM bank supports one OPEN
        # accumulation group at a time
        for j in range(2):
            for t in range(NT):
                nc.tensor.matmul(
                    cv[:, j * 256:(j + 1) * 256],
                    xvp[p][:, t, j * 128:(j + 1) * 128], wv_sb[:, t, :],
                    start=(t == 0), stop=(t == NT - 1),
                    skip_group_check=True)
        for j in range(2):
            u = p * 2 + j
            nc.vector.tensor_scalar_mul(
                out=v4[:, u, :, 0:64],
                in0=cv[:, j * 256:(j + 1) * 256].rearrange(
                    "p (h e) -> p h e", e=64),
                scalar1=m01_sb[:, u:u + 1])

    # ---- attention producer: software-pipelined over (qc, head-PAIR) ----
    state = {}

    def attn_produce(qc, pr):
        m = pr
        qsl = slice(qc * 512, (qc + 1) * 512)
        pt = {}
        for hh in range(2):
            h = pr * 2 + hh
            pt[h] = [ppool.tile([128, 4096], F32R, tag="pt",
                                name=f"pt{qc}_{h}_{half}_r{rep}")
                     for half in range(2)]
        state[(qc, pr)] = pt
        batches = [(half, b0, bsz) for half in range(2)
                   for (b0, bsz) in ((0, 3), (3, 3), (6, 2))]

        def emit_batch(i):
            half, b0, bsz = batches[i]
            sts = [big_ps.tile([128, bsz * 512], F32, tag="big",
                               name=f"st{qc}_{pr}_{half}_{b0}_{hh}_r{rep}")
                   for hh in range(2)]
            for j in range(bsz):
                k = half * 8 + b0 + j
                for hh in range(2):
                    roff = hh * 64
                    nc.tensor.matmul(
                        sts[hh][:, j * 512:(j + 1) * 512],
                        kT_sb[m][roff:roff + 64, k * 128:(k + 1) * 128],
                        qT_sb[m][roff:roff + 64, qsl],
                        start=True, stop=True)
            for hh in range(2):
                h = pr * 2 + hh
                nc.scalar.activation(
                    out=pt[h][half][:, b0 * 512:(b0 + bsz) * 512],
                    in_=sts[hh][:, 0:bsz * 512], func=EXP, scale=0.125)
        return emit_batch

    # ---- consumer: q-major ctx + denom, DVE normalize, DMA-xbar ctx^T ----
    def cons_tasks(qc, pr):
        pt = state[(qc, pr)]
        qtiles = [rpool.tile([128, 128], F32R, tag="cq",
                             name=f"cq{qc}_{pr}_{q}_r{rep}", bufs=4)
                  for q in range(4)]
        tasks = []

        def do_qsub(hh, qsub, ctile):
            h = pr * 2 + hh
            for k in range(NU):
                nc.tensor.matmul(
                    ctile[:, qsub * 128:qsub * 128 + 65],
                    pt[h][k // 8][:, (k % 8) * 512 + qsub * 128:
                                  (k % 8) * 512 + qsub * 128 + 128],
                    v_all[:, k, h * 65:(h + 1) * 65],
                    start=(k == 0), stop=(k == NU - 1),
                    skip_group_check=True)
            rec = rpool.tile([128, 1], F32, tag="rec",
                             name=f"rc{qc}_{h}_{qsub}_r{rep}", bufs=4)
            nc.vector.reciprocal(
                out=rec[:], in_=ctile[:, qsub * 128 + 64:qsub * 128 + 65])
            nc.vector.tensor_scalar_mul(
                out=qtiles[qsub][:, hh * 64:(hh + 1) * 64],
                in0=ctile[:, qsub * 128:qsub * 128 + 64], scalar1=rec[:])

        def do_head(hh):
            ctile = ctxp.tile([128, 512], F32, tag="ctx",
                              name=f"ct{qc}_{pr}_{hh}_r{rep}")
            out_tasks = []
            for qsub in range(4):
                out_tasks.append(lambda h_=hh, q_=qsub, c_=ctile:
                                 do_qsub(h_, q_, c_))
            return out_tasks

        for hh in range(2):
            tasks.extend(do_head(hh))

        def do_transposes():
            for qsub in range(4):
                qi = qc * 4 + qsub
                nc.sync.dma_start_transpose(
                    out=ctxT_sb[pr][:, qi * 128:(qi + 1) * 128],
                    in_=qtiles[qsub][:])
        tasks.append(do_transposes)
        return tasks

    def oproj_emit(qc, sc, nj, o_sb):
        qi = qc * 4 + sc
        ops = auxp.tile([128, 512], F32, tag="aux", name=f"op{qi}_{nj}_r{rep}")
        for m_ in range(2):
            nc.tensor.matmul(
                ops[:], ctxT_sb[m_][:, qi * 128:(qi + 1) * 128],
                wo_sb[:, m_, nj * 512:(nj + 1) * 512],
                start=(m_ == 0), stop=(m_ == 1))
        nc.vector.tensor_copy(
            out=o_sb[:, nj * 512:(nj + 1) * 512], in_=ops[:])
        if nj == 1:
            nc.sync.dma_start(out=out[qi * 128:(qi + 1) * 128, :], in_=o_sb[:])

    def oproj_tasks(qc):
        tasks = []
        for sc in range(4):
            box = {}

            def nj0(q_=qc, s=sc, b=box):
                b["o"] = opool.tile([128, D], F32, tag="out",
                                    name=f"o{q_}_{s}_r{rep}")
                oproj_emit(q_, s, 0, b["o"])

            def nj1(q_=qc, s=sc, b=box):
                oproj_emit(q_, s, 1, b["o"])
            tasks += [nj0, nj1]
        return tasks

    # ---- main pipeline: window ui produces unit ui, weaving V-pairs
    # (ui 0-2), consumption of unit ui-2, and dripped o-proj work into the
    # producer's batch gaps.  All V-pairs are emitted before the first cons
    # chunk (its k-loop reads every V column). ----
    units = [(qc, pr) for qc in range(NQC) for pr in range(2)]
    vq = [lambda p_=p: vpair(p_) for p in range(8)]
    odue = []
    for ui, (qc, pr) in enumerate(units):
        emit_batch = attn_produce(qc, pr)
        work = []
        if ui == 0:
            work += [lambda j_=j, h_=h: km1_half(j_, h_)
                     for j in range(4) for h in range(2)]
            # xq DMAs reuse slots: emit each only after the previous qc's
            # q-proj reads are on the books
            work += [lambda: qproj_m(0, 1), lambda: xq_dma(1)]
            work += vq[0:2]
        elif ui == 1:
            work += vq[2:7]
            work += [lambda: qproj(1)]
        elif ui == 2:
            work += [lambda: xq_dma(2)] + vq[7:8]
        elif ui in (3, 5):
            nqc = (ui + 1) // 2
            work += [lambda m_=m, q_=nqc: qproj_m(q_, m_)
                     for m in range(2)]
            if nqc == 2:
                work += [lambda: xq_dma(3)]
        if ui >= 2:
            work += cons_tasks(*units[ui - 2])
        per = (len(work) + 5) // 6 if work else 0
        for i in range(6):
            emit_batch(i)
            if per:
                for fn in work[i * per:(i + 1) * per]:
                    fn()
            if odue:
                odue.pop(0)()
        if ui >= 2 and units[ui - 2][1] == 1:
            odue += oproj_tasks(units[ui - 2][0])

    # ---- drain: consume unit 6 against the remaining o-proj backlog ----
    for ti, fn in enumerate(cons_tasks(*units[6])):
        fn()
        if odue:
            odue.pop(0)()
    for fn in odue:
        fn()

    # ---- final unit: per-q-subtile pipeline so the post-exp tail is one
    # subtile deep, not a whole unit.  h1's ctx PSUM rides a big slot (the
    # S^T pipeline is done with it). ----
    qc, pr = units[7]
    pt = state[(qc, pr)]
    # h0 ctx in the ctx bank, h1 in aux: both banks free well before the
    # last exp, so all k<14 accumulation happens during exp(u7).  The big
    # slots only free at the very end -- they host the (post-exp) transpose
    # and o-proj tiles instead, alternating so copies overlap.
    ct = [ctxp.tile([128, 512], F32, tag="ctx", name=f"ctf0_r{rep}"),
          auxp.tile([128, 512], F32, tag="aux", name=f"ctf1_r{rep}")]
    korder = [14, 15] + list(range(14))   # last-exp'd chunks first: each
    for qsub in range(4):                 # chain starts right at the gate
        qi = qc * 4 + qsub
        qtile = rpool.tile([128, 128], F32R, tag="cq",
                           name=f"cqf{qsub}_r{rep}", bufs=4)
        for hh in range(2):
            h = pr * 2 + hh
            for ki, k in enumerate(korder):
                nc.tensor.matmul(
                    ct[hh][:, qsub * 128:qsub * 128 + 65],
                    pt[h][k // 8][:, (k % 8) * 512 + qsub * 128:
                                  (k % 8) * 512 + qsub * 128 + 128],
                    v_all[:, k, h * 65:(h + 1) * 65],
                    start=(ki == 0), stop=(ki == NU - 1),
                    skip_group_check=True)
        for hh in range(2):
            h = pr * 2 + hh
            rec = rpool.tile([128, 1], F32, tag="rec",
                             name=f"rcf{h}_{qsub}_r{rep}", bufs=4)
            nc.vector.reciprocal(
                out=rec[:], in_=ct[hh][:, qsub * 128 + 64:qsub * 128 + 65])
            nc.vector.tensor_scalar_mul(
                out=qtile[:, hh * 64:(hh + 1) * 64],
                in0=ct[hh][:, qsub * 128:qsub * 128 + 64], scalar1=rec[:])
        # PE transpose (latency ~0.7us vs ~2.5us DMA-xbar roundtrip): the
        # S^T pipeline is done, so its PSUM slots are free for the dest
        tp = big_ps.tile([128, 128], F32R, tag="big", name=f"tp{qsub}_r{rep}")
        nc.tensor.matmul(tp[:], qtile[:], id_sb[:], is_transpose=True)
        # tail PSUM->SBUF copies ride the ACT engine (idle after the last
        # exp) so the DVE only carries the normalizations here
        nc.scalar.activation(
            out=ctxT_sb[pr][:, qi * 128:(qi + 1) * 128], in_=tp[:],
            func=mybir.ActivationFunctionType.Copy)
        o_sb = opool.tile([128, D], F32, tag="out", name=f"o3f_{qsub}_r{rep}")
        for nj in range(2):
            ops = big_ps.tile([128, 512], F32, tag="big",
                              name=f"opf{qi}_{nj}_r{rep}")
            for m_ in range(2):
                nc.tensor.matmul(
                    ops[:], ctxT_sb[m_][:, qi * 128:(qi + 1) * 128],
                    wo_sb[:, m_, nj * 512:(nj + 1) * 512],
                    start=(m_ == 0), stop=(m_ == 1))
            nc.scalar.activation(
                out=o_sb[:, nj * 512:(nj + 1) * 512], in_=ops[:],
                func=mybir.ActivationFunctionType.Copy)
            nc.sync.dma_start(
                out=out[qi * 128:(qi + 1) * 128, nj * 512:(nj + 1) * 512],
                in_=o_sb[:, nj * 512:(nj + 1) * 512])


def _build_program(reps=1):
    nc = bacc.Bacc("TRN2", target_bir_lowering=False, debug=False,
                   num_devices=NCORES)

    xkT = nc.dram_tensor("xkT", [D, S], F32R, kind="ExternalInput").ap()
    xqT = nc.dram_tensor("xqT", [D, S], F32R, kind="ExternalInput").ap()
    xvT = nc.dram_tensor("xvT", [D, S], F32R, kind="ExternalInput").ap()
    wqT = nc.dram_tensor("wqT", [D, DH], F32R, kind="ExternalInput").ap()
    wkT = nc.dram_tensor("wkT", [D, DH], F32R, kind="ExternalInput").ap()
    wvT = nc.dram_tensor("wvT", [D, DH], F32R, kind="ExternalInput").ap()
    woT = nc.dram_tensor("woT", [DH, D], F32R, kind="ExternalInput").ap()
    bq2 = nc.dram_tensor("bq2", [128, 2], F32, kind="ExternalInput").ap()
    bk2 = nc.dram_tensor("bk2", [128, 2], F32, kind="ExternalInput").ap()
    m01 = nc.dram_tensor("m01", [128, NU], F32, kind="ExternalInput").ap()
    ident = nc.dram_tensor("ident", [128, 128], F32R,
                           kind="ExternalInput").ap()
    out = nc.dram_tensor("out", [S, D], F32, kind="ExternalOutput").ap()
    dram = (xkT, xqT, xvT, wqT, wkT, wvT, woT, bq2, bk2, m01, ident, out)

    with tile.TileContext(nc) as tc:
        with (
            nc.allow_low_precision(
                reason="bf16 SBUF tiles; the PE truncates to fp22 at "
                       "multiply regardless"),
            tc.tile_pool(name="singles", bufs=1) as singles,
            tc.tile_pool(name="xpool", bufs=2) as xpool,
            tc.tile_pool(name="xqpool", bufs=2) as xqpool,
            tc.tile_pool(name="ppool", bufs=12) as ppool,
            tc.tile_pool(name="opool", bufs=2) as opool,
            tc.tile_pool(name="rpool", bufs=4) as rpool,
            tc.tile_pool(name="big_ps", bufs=2, space="PSUM") as big_ps,
            tc.tile_pool(name="ctx_ps", bufs=1, space="PSUM") as ctxp,
            tc.tile_pool(name="aux_ps", bufs=1, space="PSUM") as auxp,
        ):
            pools = (singles, xpool, xqpool, ppool, opool, rpool, big_ps,
                     ctxp, auxp)
            for rep in range(reps):
                _emit(nc, tc, pools, dram, rep)

    nc.compile()
    return nc


def _get_program():
    if "nc" not in _cached:
        _cached["nc"] = _build_program()
    return _cached["nc"]


def kernel(query, key, value, mask, Wq, bq, Wk, bk, Wv, bv, Wo, bo):
    query = np.asarray(query, dtype=np.float32)
    key = np.asarray(key, dtype=np.float32)
    value = np.asarray(value, dtype=np.float32)
    mask = np.asarray(mask)
    Wq, bq = np.asarray(Wq, dtype=np.float32), np.asarray(bq, dtype=np.float32)
    Wk, bk = np.asarray(Wk, dtype=np.float32), np.asarray(bk, dtype=np.float32)
    Wv, bv = np.asarray(Wv, dtype=np.float32), np.asarray(bv, dtype=np.float32)
    Wo, bo = np.asarray(Wo, dtype=np.float32), np.asarray(bo, dtype=np.float32)

    nc = _get_program()

    c = np.ascontiguousarray
    in_maps = []
    for core in range(NCORES):
        b, g = core // G, core % G
        sl = slice(g * DH, (g + 1) * DH)
        mk = (mask[b, 0, 0, :] != 0).astype(np.float32)
        bf = ml_dtypes.bfloat16
        in_maps.append({
            "xqT": c(query[b].T).astype(bf), "xkT": c(key[b].T).astype(bf),
            "xvT": c(value[b].T).astype(bf),
            "wqT": c(Wq[sl, :].T).astype(bf), "wkT": c(Wk[sl, :].T).astype(bf),
            "wvT": c(Wv[sl, :].T).astype(bf),
            "woT": c(Wo[:, sl].T).astype(bf),
            "bq2": c(bq[sl].reshape(2, 128).T), "bk2": c(bk[sl].reshape(2, 128).T),
            "m01": c(mk.reshape(NU, 128).T),
            "ident": np.eye(128, dtype=bf),
        })

    res = run_bass_kernel_spmd(nc, in_maps, core_ids=list(range(NCORES)))
    _cached["last_results"] = res

    # value-bias folds into the output bias: sum_k attn*(v+bv) = ctx + bv
    bo_eff = bo + bv @ Wo.T
    result = np.empty((B, S, D), dtype=np.float32)
    for b in range(B):
        acc = res.results[b * G + 0]["out"].copy()
        for g in range(1, G):
            acc += res.results[b * G + g]["out"]
        result[b] = acc + bo_eff
    return result
